# revision 36
# baseline (speedup 1.0000x reference)
"""Distributed Trainium2 Bass kernel: RMSNorm + 16-head attention + out-proj.

Problem (hardcoded): tokens [4, 2048, 2048] f32, DIM=2048, HEADS=16, DHEAD=128.
Sharding: 8 cores = 4 batches x 2 sequence halves. Each core owns 1024 query
rows of one batch; K/V for the full 2048 keys arrive via pair-wise AllGathers
(cores 2i and 2i+1 hold the two halves of batch i). All matmuls run in bf16
with fp32 PSUM accumulation; the rmsnorm statistics are computed in fp32 and
folded into a PE transpose (x.T @ diag(rstd)).

Host-side folding: norm_weight and the q-scale (dhead^-0.5) are folded into
the projection weight matrices.

Queue discipline: weight DMAs go on nc.sync / nc.scalar (HWDGE) and complete
before the collectives; the gather-dependent attention loads go on nc.gpsimd
(SWDGE rings) so a wait on the collective semaphore never blocks anything
through shared HWDGE ring flow-control.
"""

import os
import sys

for p in ("/opt/trn_rl_repo", "/root/.axon_site/_ro/trn_rl_repo"):
    if os.path.isdir(p) and p not in sys.path:
        sys.path.append(p)

import numpy as np
import ml_dtypes

BF16 = ml_dtypes.bfloat16

DIM = 2048
HEADS = 16
DHEAD = 128
B = 4
N = 2048
NCORES = 8
LOCAL = N // 2          # 1024 rows per core
P = 128                 # partitions
MC = DIM // P           # 16 model chunks
RT = LOCAL // P         # 8 row tiles
QH = 2                  # qrow halves of 512
FD = 512                # moving free dim
KCHUNKS = N // P        # 16 key chunks (over both blocks)
OC = DIM // FD          # 4 out-col chunks
EPS = 1.1920929e-07

_CACHED_NC = None


def build():
    from concourse import bacc, tile, mybir
    from concourse.masks import make_identity

    fp32 = mybir.dt.float32
    bf16 = mybir.dt.bfloat16

    nc = bacc.Bacc(
        "TRN2", target_bir_lowering=False, debug=False, num_devices=NCORES
    )

    toks = nc.dram_tensor("tokens", [LOCAL, DIM], fp32, kind="ExternalInput")
    wq = nc.dram_tensor("wq", [HEADS, P, MC, P], bf16, kind="ExternalInput")
    wk = nc.dram_tensor("wk", [HEADS, P, MC, P], bf16, kind="ExternalInput")
    wv = nc.dram_tensor("wv", [DIM, DIM], bf16, kind="ExternalInput")
    wo = nc.dram_tensor("wo", [DIM, DIM], bf16, kind="ExternalInput")
    out = nc.dram_tensor("out", [LOCAL, DIM], fp32, kind="ExternalOutput")

    # internal DRAM scratch
    kl_d = nc.dram_tensor("kl_d", [HEADS, P, LOCAL], bf16)   # local k^T
    vl_h = [nc.dram_tensor(f"vl{g}", [RT, P, DIM // 2], bf16)
            for g in range(2)]                                # local v, h-group
    kg_d = nc.dram_tensor("kg_d", [2, HEADS, P, LOCAL], bf16)
    vg_h = [nc.dram_tensor(f"vg{g}", [2, RT, P, DIM // 2], bf16)
            for g in range(2)]

    klv = kl_d.ap()
    kgv = [kg_d[b] for b in range(2)]

    RG = [[0, 1], [2, 3], [4, 5], [6, 7]]

    with tile.TileContext(nc) as tc:
      with tc.tile_pool(name="persist", bufs=1) as persist:
        qt_sb = [persist.tile([P, LOCAL], bf16, tag=f"qt{h}", name=f"qt{h}")
                 for h in range(HEADS)]
        avt_sb = [persist.tile([P, LOCAL], bf16, tag=f"avt{h}", name=f"avt{h}")
                  for h in range(HEADS)]
        ones_sb = persist.tile([P, 1], bf16, tag="ones")
        nc.vector.memset(ones_sb[:], 1.0)
        eps_sb = persist.tile([P, 1], fp32, tag="eps")
        nc.vector.memset(eps_sb[:], EPS)

        # ------- Phase 1: rmsnorm fused into a PE transpose (x.T @ diag) ----
        with tc.tile_pool(name="xt", bufs=1) as xtp:
            xT = [xtp.tile([P, LOCAL], bf16, tag=f"xt{m}", name=f"xt{m}")
                  for m in range(MC)]
            with (
                tc.tile_pool(name="p1", bufs=3) as p1,
                tc.tile_pool(name="stat", bufs=4) as stat,
                tc.tile_pool(name="psT", bufs=4, space="PSUM") as psT,
            ):
                ident = p1.tile([P, P], bf16, tag="ident", bufs=1)
                make_identity(nc, ident[:])
                for rt in range(RT):
                    x = p1.tile([P, DIM], fp32, tag="x")
                    nc.sync.dma_start(out=x[:], in_=toks[rt * P:(rt + 1) * P, :])
                    sq = p1.tile([P, DIM], fp32, tag="sq")
                    ssq = stat.tile([P, 1], fp32, tag="ssq")
                    nc.scalar.activation(
                        sq[:], x[:], mybir.ActivationFunctionType.Square,
                        accum_out=ssq[:],
                    )
                    std = stat.tile([P, 1], fp32, tag="std")
                    nc.scalar.activation(
                        std[:], ssq[:], mybir.ActivationFunctionType.Sqrt,
                        bias=eps_sb[:], scale=1.0 / DIM,
                    )
                    rstd = stat.tile([P, 1], fp32, tag="rstd")
                    nc.vector.reciprocal(rstd[:], std[:])
                    diag = stat.tile([P, P], bf16, tag="diag")
                    nc.vector.tensor_scalar_mul(diag[:], ident[:], rstd[:])
                    xb = p1.tile([P, DIM], bf16, tag="xb")
                    nc.scalar.activation(
                        xb[:], x[:], mybir.ActivationFunctionType.Copy
                    )
                    # xT[m][:, rows] = (x[:, m-chunk]).T @ diag(rstd)
                    for m in range(MC):
                        pt = psT.tile([P, P], fp32, tag="pt")
                        nc.tensor.matmul(
                            pt[:], xb[:, m * P:(m + 1) * P], diag[:],
                            start=True, stop=True,
                        )
                        nc.vector.tensor_copy(
                            xT[m][:, rt * P:(rt + 1) * P], pt[:]
                        )

            # ---------------- Phase 2: k^T and v projections ---------------
            with (
                tc.tile_pool(name="p2", bufs=2) as p2,
                tc.tile_pool(name="psA", bufs=3, space="PSUM") as psA,
            ):
                wv_full = p2.tile([P, MC, DIM], bf16, tag="wvf", bufs=1)
                nc.scalar.dma_start(
                    out=wv_full[:],
                    in_=wv.ap().rearrange("(mc p) d -> p mc d", p=P),
                )
                # k^T: stationary = wk chunks, moving = xT
                for h in range(HEADS):
                    wk_sb = p2.tile([P, MC, P], bf16, tag="wk")
                    nc.sync.dma_start(out=wk_sb[:], in_=wk[h])
                    for q in range(QH):
                        ps = psA.tile([P, FD], fp32, tag="pp", bufs=4)
                        for m in range(MC):
                            nc.tensor.matmul(
                                ps[:], wk_sb[:, m, :],
                                xT[m][:, q * FD:(q + 1) * FD],
                                start=(m == 0), stop=(m == MC - 1),
                            )
                        stg = p2.tile([P, FD], bf16, tag="stg", bufs=8)
                        nc.vector.tensor_copy(stg[:], ps[:])
                        nc.sync.dma_start(
                            out=klv[h][:, q * FD:(q + 1) * FD], in_=stg[:]
                        )
                # k-gather starts while v/q projections run
                nc.gpsimd.collective_compute(
                    "AllGather", mybir.AluOpType.bypass,
                    replica_groups=RG,
                    ins=[kl_d.ap().opt()],
                    outs=[kg_d.ap().opt()],
                )
                # v: stationary = xT chunks, moving = wv_full[mc, oc] slices
                # gathered per head-group so attention h<8 starts sooner
                for g in range(2):
                    for oc in (2 * g, 2 * g + 1):
                        for rt in range(RT):
                            ps = psA.tile([P, FD], fp32, tag="pp", bufs=4)
                            for m in range(MC):
                                nc.tensor.matmul(
                                    ps[:], xT[m][:, rt * P:(rt + 1) * P],
                                    wv_full[:, m, oc * FD:(oc + 1) * FD],
                                    start=(m == 0), stop=(m == MC - 1),
                                )
                            stg = p2.tile([P, FD], bf16, tag="stg", bufs=8)
                            nc.vector.tensor_copy(stg[:], ps[:])
                            nc.sync.dma_start(
                                out=vl_h[g].ap()[rt][
                                    :, (oc % 2) * FD:(oc % 2 + 1) * FD
                                ],
                                in_=stg[:],
                            )
                    nc.gpsimd.collective_compute(
                        "AllGather", mybir.AluOpType.bypass,
                        replica_groups=RG,
                        ins=[vl_h[g].ap().opt()],
                        outs=[vg_h[g].ap().opt()],
                    )

                # ---------------- Phase 3: q^T projection ------------------
                for h in range(HEADS):
                    wq_sb = p2.tile([P, MC, P], bf16, tag="wk")
                    nc.gpsimd.dma_start(out=wq_sb[:], in_=wq[h])
                    for q in range(QH):
                        ps = psA.tile([P, FD], fp32, tag="pp", bufs=4)
                        for m in range(MC):
                            nc.tensor.matmul(
                                ps[:], wq_sb[:, m, :],
                                xT[m][:, q * FD:(q + 1) * FD],
                                start=(m == 0), stop=(m == MC - 1),
                            )
                        nc.vector.tensor_copy(
                            qt_sb[h][:, q * FD:(q + 1) * FD], ps[:]
                        )

        # ---------------- Phase 4: attention -------------------------------
        with tc.tile_pool(name="proj", bufs=2) as pr:
            wo_sb = pr.tile([P, MC, DIM], bf16, tag="wo", bufs=1)
            nc.sync.dma_start(
                out=wo_sb[:],
                in_=wo.ap().rearrange("(hh p) d -> p hh d", p=P),
            )
            with (
                tc.tile_pool(name="attn", bufs=2) as ap_,
                tc.tile_pool(name="psB", bufs=2, space="PSUM") as psB,
            ):
                for h in range(HEADS):
                    kt_sb = ap_.tile([P, N], bf16, tag="kt", bufs=2)
                    for b in range(2):
                        nc.gpsimd.dma_start(
                            out=kt_sb[:, b * LOCAL:(b + 1) * LOCAL], in_=kgv[b][h]
                        )
                    v_sb = ap_.tile([P, KCHUNKS, P], bf16, tag="vt", bufs=2)
                    hc = (h % 8) * DHEAD
                    for b in range(2):
                        nc.gpsimd.dma_start(
                            out=v_sb[:, b * RT:(b + 1) * RT, :],
                            in_=vg_h[h // 8][b][:, :, hc:hc + DHEAD].rearrange(
                                "r p d -> p r d"
                            ),
                        )
                    for q in range(QH):
                        av = psB.tile([P, FD], fp32, tag="av", bufs=1, name="av")
                        rs = psB.tile([1, FD], fp32, tag="rs", bufs=1, name="rs")
                        for kp in range(KCHUNKS // 2):
                            sim = psB.tile([P, 2, FD], fp32, tag="sim", bufs=3)
                            for j in range(2):
                                kc = kp * 2 + j
                                nc.tensor.matmul(
                                    sim[:, j, :], kt_sb[:, kc * P:(kc + 1) * P],
                                    qt_sb[h][:, q * FD:(q + 1) * FD],
                                    start=True, stop=True,
                                )
                            pT = ap_.tile([P, 2, FD], bf16, tag="pT", bufs=4)
                            nc.scalar.activation(
                                pT[:], sim[:], mybir.ActivationFunctionType.Exp
                            )
                            for j in range(2):
                                kc = kp * 2 + j
                                nc.tensor.matmul(
                                    av[:], v_sb[:, kc, :], pT[:, j, :],
                                    start=(kc == 0), stop=(kc == KCHUNKS - 1),
                                )
                            for j in range(2):
                                kc = kp * 2 + j
                                nc.tensor.matmul(
                                    rs[:], ones_sb[:], pT[:, j, :],
                                    start=(kc == 0), stop=(kc == KCHUNKS - 1),
                                )
                        rc1 = ap_.tile([1, FD], fp32, tag="rc1")
                        nc.vector.reciprocal(rc1[:], rs[:])
                        rcb = ap_.tile([P, FD], fp32, tag="rcb")
                        nc.gpsimd.partition_broadcast(rcb[:], rc1[:])
                        nc.vector.tensor_mul(
                            avt_sb[h][:, q * FD:(q + 1) * FD], av[:], rcb[:]
                        )

            # ---------------- Phase 5: output projection -------------------
            with tc.tile_pool(name="psC", bufs=2, space="PSUM") as psC:
                for qt in range(RT):
                    ps = psC.tile([P, OC, FD], fp32, tag="po")
                    for h in range(HEADS):
                        for oc in range(OC):
                            nc.tensor.matmul(
                                ps[:, oc, :], avt_sb[h][:, qt * P:(qt + 1) * P],
                                wo_sb[:, h, oc * FD:(oc + 1) * FD],
                                start=(h == 0), stop=(h == HEADS - 1),
                            )
                    for oc in range(OC):
                        ostg = pr.tile([P, FD], fp32, tag="ostg")
                        nc.vector.tensor_copy(ostg[:], ps[:, oc, :])
                        nc.sync.dma_start(
                            out=out[qt * P:(qt + 1) * P, oc * FD:(oc + 1) * FD],
                            in_=ostg[:],
                        )

    nc.compile()
    return nc


def _get_nc():
    global _CACHED_NC
    if _CACHED_NC is None:
        _CACHED_NC = build()
    return _CACHED_NC


def _make_in_maps(tokens, norm_weight, w_q, w_kv, w_out):
    tokens = np.asarray(tokens, dtype=np.float32)
    norm_weight = np.asarray(norm_weight, dtype=np.float32)
    w_q = np.asarray(w_q, dtype=np.float32)
    w_kv = np.asarray(w_kv, dtype=np.float32)
    w_out = np.asarray(w_out, dtype=np.float32)

    wq_eff = (w_q * norm_weight[:, None]) * (DHEAD ** -0.5)
    wk_eff = w_kv[:, :DIM] * norm_weight[:, None]
    wv_eff = w_kv[:, DIM:] * norm_weight[:, None]

    def pack_T(w):  # [DIM, DIM] -> [h, p, mc, d]
        t = w.reshape(MC, P, HEADS, DHEAD)
        return np.ascontiguousarray(t.transpose(2, 1, 0, 3)).astype(BF16)

    wq_p = pack_T(wq_eff)
    wk_p = pack_T(wk_eff)
    wv_b = wv_eff.astype(BF16)
    wo_b = w_out.astype(BF16)

    in_maps = []
    for c in range(NCORES):
        bi, hi = c // 2, c % 2
        tk = np.ascontiguousarray(tokens[bi, hi * LOCAL:(hi + 1) * LOCAL])
        in_maps.append(
            {"tokens": tk, "wq": wq_p, "wk": wk_p, "wv": wv_b, "wo": wo_b}
        )
    return in_maps


def _assemble(results):
    out = np.empty((B, N, DIM), np.float32)
    for c in range(NCORES):
        bi, hi = c // 2, c % 2
        out[bi, hi * LOCAL:(hi + 1) * LOCAL] = results[c]["out"]
    return out


def run(trace=False, tmpdir=None, **inputs):
    from concourse.bass_utils import run_bass_kernel_spmd

    nc = _get_nc()
    in_maps = _make_in_maps(**inputs)
    res = run_bass_kernel_spmd(
        nc, in_maps, core_ids=list(range(NCORES)), trace=trace, tmpdir=tmpdir
    )
    return _assemble(res.results), res


def kernel(**inputs):
    out, _ = run(trace=False, **inputs)
    return out
